# revision 34
# baseline (speedup 1.0000x reference)
"""Trainium2 Bass kernel for nn_EquShiftQ2DF (dense_cnn).

Data-parallel over 8 NeuronCores: each core processes 16 of the 128 samples
for the conv/ih/df paths; all conv weights are replicated (host pre-arranges
each weight into the exact SBUF tile layout so every DMA is a contiguous 2D
copy).

The big es FC1 (16384->1024, 67MB of weights) is K-SHARDED across the 8
cores: core c takes K-rows [c*2048, (c+1)*2048) of es_w1 for ALL 128 samples,
computes a partial es1 [128, 1024] in PSUM, and a ReduceScatter(add) over the
sample dim hands each core its own 16 samples' finished es1 [16, 1024].  This
cuts the per-core es1 weight stream 8x and the PE row count 8x.

SKEW ROBUSTNESS: the 8 cores launch with 20-100us of random skew (measured),
so the RS completes at an unpredictable wall time.  Everything that consumes
the RS result (es1 bias/relu, es1T transposes, es2 chain, the es-half of the
df contraction) is pushed as late as possible: the df contraction is split
into its ih-half (RS-independent, runs early) and es-half (late), and the RS
receive is issued under tc.tile_wait_until(0.25) so the Tile scheduler places
it late in every engine's program instead of head-of-line-blocking the sync
DMA queue / vector queue while the collective drains.

Per-core pipeline (S=16 samples):
  es branch   : K-sharded FC1 (above) -> FC(1024->512) sample-local.
  ih branch   : conv s2 x2 (as 9-offset shifted matmuls) + FC(4608->512).
  enc convs   : conv1 (host im2col, K=9), conv2/conv3 as 9-offset shifted
                matmuls with K=ci chunks accumulated in PSUM, 2x2 maxpool on
                DVE, pc conv (stride 2; c3a stored position-major so pc's
                moving AP has contiguous 16-sample runs).
  hypernet    : df FC -> per-sample filter coeffs; dynamic-filter contraction
                via a fixed-basis matmul (t = basis^T @ H) plus per-sample
                [128x8]x[128x8] matmuls (bf16); layout shuffles via two DRAM
                bounces batched into 1-2 DMA instructions each.

Engine balance: conv drains are split scalar/vector; conv3's PSUM rotates
through 5 banks (3 'mm' + the dead es1 accumulator banks) so the ~2.4us
strided drains never stall the next oc; RS-dependent SBUF-only epilogues run
on the otherwise-idle gpsimd engine.

Per-stage dtype is configurable: 'bf16' | 'f32r' | 'f32'.  bf16 halves the
HBM weight stream AND halves the PE stationary-load (LDWEIGHTS) time; the
dynamic-filter tail (basis/H/t/L/R/c2) also runs bf16 (adds ~1.3e-3 rel err,
well under the 2e-2 gate).
"""
import numpy as np
import ml_dtypes
from contextlib import ExitStack

import concourse.bass as bass
import concourse.tile as tile
from concourse import bacc, mybir
from concourse.bass_utils import run_bass_kernel_spmd
from concourse.masks import make_identity

NCORES = 8
B = 128
S = B // NCORES  # samples per core
KSH = 16384 // NCORES  # es1 K-shard rows per core

DEFAULT_CFG = {"conv": "bf16", "fc": "bf16", "pc": "bf16"}

_DT_DRAM = {"bf16": mybir.dt.bfloat16, "f32r": mybir.dt.float32r, "f32": mybir.dt.float32}
_DT_ACT = {"bf16": mybir.dt.bfloat16, "f32r": mybir.dt.float32r, "f32": mybir.dt.float32}
_DT_NP = {"bf16": ml_dtypes.bfloat16, "f32r": np.float32, "f32": np.float32}

F32 = mybir.dt.float32


def _mm(ap, key, cfg):
    """Cast an activation AP to the stage's matmul dtype."""
    if cfg[key] == "f32r" and ap.dtype == F32:
        return ap.bitcast(mybir.dt.float32r)
    return ap


def build(cfg):
    nc = bacc.Bacc("TRN2", target_bir_lowering=False, debug=False, num_devices=NCORES)
    cdt = _DT_DRAM[cfg["conv"]]
    fdt = _DT_DRAM[cfg["fc"]]
    pdt = _DT_DRAM[cfg["pc"]]
    cat = _DT_ACT[cfg["conv"]]
    fat = _DT_ACT[cfg["fc"]]
    pat = _DT_ACT[cfg["pc"]]

    D = nc.dram_tensor
    # per-core activations
    obsT_d = D("obsT", [128, KSH // 128, B], fdt, kind="ExternalInput")  # [p, kc, s]
    im1_d = D("im1", [9, S, 484], cdt, kind="ExternalInput")
    ihim_d = D("ihim", [9, S, 144], cdt, kind="ExternalInput")
    # replicated weights (host pre-arranged to tile layouts)
    w1r_d = D("w1r", [9, 256], cdt, kind="ExternalInput")
    w2t_d = D("w2t", [128, 4, 2, 9, 128], cdt, kind="ExternalInput")
    w3t_d = D("w3t", [8, 128, 4, 9, 128], cdt, kind="ExternalInput")
    pcwt_d = D("pcwt", [128, 8, 9, 64], pdt, kind="ExternalInput")
    ihw1r_d = D("ihw1r", [9, 64], cdt, kind="ExternalInput")
    ihw2t_d = D("ihw2t", [64, 9, 128], cdt, kind="ExternalInput")
    ihfcwt_d = D("ihfcwt", [4608, 512], fdt, kind="ExternalInput")
    esw1t_d = D("esw1t", [KSH, 1024], fdt, kind="ExternalInput")  # per-core K-shard
    esw2t_d = D("esw2t", [1024, 512], fdt, kind="ExternalInput")
    dfwt_d = D("dfwt", [1024, 2056], fdt, kind="ExternalInput")
    bprime_d = D("bprime", [72, 256], mybir.dt.bfloat16, kind="ExternalInput")
    c2wt_d = D("c2wt", [8, 8, 16], mybir.dt.bfloat16, kind="ExternalInput")
    # all conv biases packed per-partition: [b1(2) b2(4) b3(8) ihb2(1) pcb(1) ihb1(1)]
    bblobA_d = D("bblobA", [128, 17], F32, kind="ExternalInput")
    # all S-broadcast FC biases: [esb1(1024) esb2(512) ihfcb(512) c2b(16) dfb(2056)]
    bblobB_d = D("bblobB", [S, 4120], F32, kind="ExternalInput")
    out_d = D("out", [S, 2, 8], F32, kind="ExternalOutput")
    scr_h = D("scr_h", [8, 9, 8, S], mybir.dt.bfloat16)   # [e, kk, g, s]
    scr_t = D("scr_t", [256, 8 * S], mybir.dt.bfloat16)   # [(c,d), (g,s)]
    CCDT = mybir.dt.bfloat16  # RS payload dtype (halves fabric bytes)
    cc_in = D("es1cc_in", [B, 1024], CCDT)   # partial es1
    cc_out = D("es1cc_out", [S, 1024], CCDT)  # post-RS

    RELU = mybir.ActivationFunctionType.Relu

    with tile.TileContext(nc) as tc, ExitStack() as ctx:
        wts = ctx.enter_context(tc.tile_pool(name="wts", bufs=1))
        stream = ctx.enter_context(tc.tile_pool(name="stream", bufs=3))
        acts = ctx.enter_context(tc.tile_pool(name="acts", bufs=1))
        work = ctx.enter_context(tc.tile_pool(name="work", bufs=2))
        ps = ctx.enter_context(tc.tile_pool(name="ps", bufs=1, space="PSUM"))

        dma = nc.sync.dma_start

        # ---- phase 0: constants / weights; issue order = DMA queue order,
        # so first-needed tensors go first (conv1+ih1 inputs, then conv2) ----
        w1r_t = wts.tile([9, 256], cdt)
        dma(out=w1r_t, in_=w1r_d.ap())
        # conv1 input immediately behind its weights: first matmul at ~3us
        im1_t = wts.tile([9, S, 484], cdt)
        dma(out=im1_t[:, 0:1, :], in_=im1_d.ap()[:, 0:1, :])
        ihw1r_t = wts.tile([9, 64], cdt)
        dma(out=ihw1r_t, in_=ihw1r_d.ap())
        # conv biases, one packed DMA
        bblobA = wts.tile([128, 17], F32)
        dma(out=bblobA, in_=bblobA_d.ap())
        b1t = bblobA[:, 0:2]
        b2t = bblobA[:, 2:6]
        b3t = bblobA[:, 6:14]
        ihb2t = bblobA[:, 14:15]
        pcbt = bblobA[0:64, 15:16]
        ihb1t = bblobA[0:64, 16:17]
        # whole im2col inputs (small): no per-sample DMAs in phase 1
        ihim_t = wts.tile([9, S, 144], cdt)
        dma(out=ihim_t, in_=ihim_d.ap())
        dma(out=im1_t[:, 1:, :], in_=im1_d.ap()[:, 1:, :])
        obsT_t = wts.tile([128, KSH // 128, B], fdt)
        dma(out=obsT_t, in_=obsT_d.ap())
        ihw2t_t = wts.tile([64, 9, 128], cdt)
        dma(out=ihw2t_t, in_=ihw2t_d.ap())
        # conv2 weights: oc0 up-front (needed at phase-1 start); oc1-3 plus
        # cold constants are issued after phase 0 so the es1 stream isn't
        # stuck behind them in the DMA queue.
        w2t_t = wts.tile([128, 4, 2, 9, 128], cdt)
        dma(out=w2t_t[:, 0, :, :, :], in_=w2t_d.ap()[:, 0, :, :, :])
        bblobB = wts.tile([S, 4120], F32)
        dma(out=bblobB, in_=bblobB_d.ap())
        esb1bc = bblobB[:, 0:1024]
        esb2bc = bblobB[:, 1024:1536]
        ihfcbc = bblobB[:, 1536:2048]
        c2bbc = bblobB[:, 2048:2064]
        dfbbc = bblobB[:, 2064:4120]
        bprime_t = wts.tile([72, 256], mybir.dt.bfloat16)
        c2wt_t = wts.tile([8, 8, 16], mybir.dt.bfloat16)
        ident = wts.tile([16, 16], F32)
        make_identity(nc, ident)

        def deferred_const_dmas():
            for oc in range(1, 4):
                dma(out=w2t_t[:, oc, :, :, :], in_=w2t_d.ap()[:, oc, :, :, :])
            dma(out=bprime_t, in_=bprime_d.ap())
            dma(out=c2wt_t, in_=c2wt_d.ap())

        # persistent activations
        pooled = acts.tile([128, 4, S, 100], cat)
        pad1 = acts.tile([64, S, 14, 14], cat)
        nc.vector.memset(pad1.bitcast(F32) if pad1.dtype == mybir.dt.float32r else pad1, 0.0)
        ih2act = acts.tile([128, S, 36], fat)
        es1sb = acts.tile([S, 1024], F32)
        es1T = acts.tile([128, 8, S], fat)
        es2T = acts.tile([128, 4, S], fat)
        ihT = acts.tile([128, 4, S], fat)
        wb_sb = acts.tile([S, 2056], F32)
        L_sb = acts.tile([128, 2, 8, S], mybir.dt.bfloat16)
        dbias_t = acts.tile([8, S], F32)
        bb = acts.tile([8, 8, S], F32)

        es1_ps = []
        for h in range(2):
            es1_ps.append(ps.tile([B, 512], F32, tag=f"acc{h}", bufs=1, name=f"es1ps{h}"))

        # ---------- drip-feed unit queue: DMA-heavy FC work interleaved ----------
        # Units are (dma_fn, mm_fn) pairs; the dma side runs LOOK units ahead
        # of the matmul side so a dripped matmul never waits on its own DMA.
        from collections import deque
        dma_q = deque()
        mm_q = deque()
        LOOK = 2
        dstate = {"d": 0, "m": 0}

        def add_unit(dma_fn, mm_fn):
            dma_q.append(dma_fn)
            mm_q.append(mm_fn)

        def drip(n, keep=0):
            for _ in range(n):
                if len(mm_q) <= keep:
                    return
                while dma_q and dstate["d"] < dstate["m"] + 1 + LOOK:
                    fn = dma_q.popleft()
                    dstate["d"] += 1
                    if fn:
                        fn()
                f = mm_q.popleft()
                dstate["m"] += 1
                f()

        def units_len():
            return len(mm_q)

        NK2 = KSH // 256  # es1 double-K-chunks per core (8)

        def es1_unit(kc2):
            # two 128-row K-chunks per unit: halves the sync-queue DMA count
            # so the es1 weight stream lands early enough to trigger the
            # ReduceScatter by ~35us instead of ~70us.
            cell = {}
            def dmaf():
                rw = stream.tile([128, 2, 1024], fdt, tag="esw1", bufs=3,
                                 name="esw1t")
                src = esw1t_d.ap()[kc2 * 256:(kc2 + 1) * 256, :]
                dma(out=rw, in_=src.rearrange("(c p) n -> p c n", c=2))
                cell["rw"] = rw
            def mmf():
                rw = cell["rw"]
                for c in range(2):
                    kc = 2 * kc2 + c
                    for h in range(2):
                        nc.tensor.matmul(es1_ps[h], _mm(obsT_t[:, kc, :], "fc", cfg),
                                         rw[:, c, h * 512:(h + 1) * 512],
                                         start=(kc == 0), stop=(kc == 2 * NK2 - 1),
                                         skip_group_check=True)
            return dmaf, mmf

        for kc2 in range(NK2):
            add_unit(*es1_unit(kc2))

        box = {}

        def es1_fin():
            # partial [128, 1024] -> DRAM -> ReduceScatter(add, sample dim).
            # Trigger only: the receive DMA + bias/relu live in es1_recv, queued
            # ~40 units later so the RS-gated DMA never heads the sync queue
            # (head-of-line there stalled the whole FC weight stream ~37us).
            part = work.tile([B, 1024], CCDT, tag="es1part", bufs=1, name="es1part")
            for h in range(2):
                nc.vector.tensor_copy(part[:, h * 512:(h + 1) * 512], es1_ps[h])
            dma(out=cc_in.ap(), in_=part)
            nc.gpsimd.collective_compute(
                "ReduceScatter", mybir.AluOpType.add,
                replica_groups=[list(range(NCORES))],
                ins=[cc_in.ap().opt()], outs=[cc_out.ap().opt()])

        def es1_recv():
            # tile_wait_until: the scheduler otherwise places this DMA early
            # in the sync program (its model has zero cross-core skew), and a
            # late-arriving ReduceScatter then head-of-line-blocks the whole
            # weight stream.  Forcing sim-readiness to 250us pushes it (and
            # the dependent es chain) late in every engine's program.
            with tc.tile_wait_until(0.25):
                pre = work.tile([S, 1024], CCDT, tag="es1pre", bufs=1, name="es1pre")
                dma(out=pre, in_=cc_out.ap())
                nc.vector.tensor_add(es1sb, pre, esb1bc)
                nc.vector.tensor_relu(es1sb, es1sb)

        add_unit(None, es1_fin)

        # Transposes ping-pong through the (dead-by-then) es1 PSUM banks so
        # they don't fight the fc accumulators for bufs (was serializing each
        # transpose behind the previous vector copy, ~322ns apiece).
        tp_state = {"i": 0}

        def tp_tile(shape):
            # fc-only ping-pong; acc0/acc1 are lent to conv3's p3 rotation
            return ps.tile(shape, F32, tag="fc", bufs=2, name="tpos")

        def transpose_unit(src_fn, dst_fn):
            def f():
                tp = tp_tile([128, S])
                nc.tensor.transpose(tp, src_fn(), ident)
                nc.vector.tensor_copy(dst_fn(), tp)
            return f

        # es2/df-es weight streams are prefetched by dma-only units queued
        # BEFORE es1_recv: the recv carries tile_wait_until(0.25), and any DMA
        # with a higher build priority would otherwise be scheduled after it
        # on the sync queue, starving the late es chain (~13us PE stall).
        es2_rws = []

        def es2_dma_all():
            # 2 batched DMAs (was 8): the prefetch burst was congesting the
            # sync queue (~0.6us serial issue each) right when phase1's
            # on-demand weight stream needed it.
            for half in range(2):
                rw = stream.tile([128, 4, 512], fdt, tag="esw2", bufs=2,
                                 name="esw2t")
                s4 = esw2t_d.ap()[half * 512:(half + 1) * 512, :]
                dma(out=rw, in_=s4.rearrange("(c p) n -> p c n", c=4))
                es2_rws.append(rw)

        def es2_unit(c):
            def mmf():
                if c == 0:
                    box["p_es2"] = ps.tile([S, 512], F32, tag="fc", bufs=2,
                                           name="pes2")
                nc.tensor.matmul(box["p_es2"], _mm(es1T[:, c, :], "fc", cfg),
                                 es2_rws[c // 4][:, c % 4, :],
                                 start=(c == 0), stop=(c == 7),
                                 skip_group_check=True)
            return None, mmf

        def es2_fin():
            es2sb = work.tile([S, 512], F32, tag="fcout")
            nc.vector.tensor_add(es2sb, box["p_es2"], esb2bc)
            nc.vector.tensor_relu(es2sb, es2sb)
            box["es2sb"] = es2sb

        def ihfc_unit(sp):
            cell = {}
            def dmaf():
                rw = stream.tile([128, 512], fdt, tag="ihfcw", name="ihfcwt")
                dma(out=rw, in_=ihfcwt_d.ap()[sp * 128:(sp + 1) * 128, :])
                cell["rw"] = rw
            def mmf():
                if sp == 0:
                    box["p_ihfc"] = ps.tile([S, 512], F32, tag="fc", bufs=2,
                                            name="pihfc")
                nc.tensor.matmul(box["p_ihfc"], _mm(ih2act[:, :, sp], "fc", cfg),
                                 cell["rw"], start=(sp == 0), stop=(sp == 35),
                                 skip_group_check=True)
            return dmaf, mmf

        def ihfc_fin():
            ihsb = work.tile([S, 512], F32, tag="fcout")
            nc.vector.tensor_add(ihsb, box["p_ihfc"], ihfcbc)
            nc.vector.tensor_relu(ihsb, ihsb)
            box["ihsb"] = ihsb

        nsl = [(0, 512), (512, 512), (1024, 512), (1536, 512), (2048, 8)]
        # df contraction split: ih half (c=4..7, RS-independent) runs early,
        # es half (c=0..3) late.  This pushes every RS-dependent unit as late
        # as possible so cross-core launch skew (measured 20-60us) is absorbed
        # instead of stalling the PE at the ReduceScatter.
        wbI = acts.tile([S, 5, 512], F32)

        dfes_rws = {}

        def dfes_dma(ni):
            n0, nsz = nsl[ni]
            def dmaf():
                rw = stream.tile([128, 4, 512], fdt, tag="dfwes", bufs=5,
                                 name="dfwes_t")
                s4 = dfwt_d.ap()[0:512, n0:n0 + nsz]
                dma(out=rw[:, :, 0:nsz],
                    in_=s4.rearrange("(c p) n -> p c n", c=4))
                dfes_rws[ni] = rw
            return dmaf

        def df_unit(ni, c):
            n0, nsz = nsl[ni]
            key = "pwih" if c >= 4 else "pw"
            cell = {}
            def dmaf():
                rw = stream.tile([128, 512], fdt, tag="dfw", name="dfwt_t")
                dma(out=rw[:, 0:nsz], in_=dfwt_d.ap()[c * 128:(c + 1) * 128, n0:n0 + nsz])
                cell["rw"] = rw
            def mmf():
                if c % 4 == 0:
                    box[key] = ps.tile([S, 512], F32, tag="fc", bufs=2, name=key)
                lhsT = es2T[:, c, :] if c < 4 else ihT[:, c - 4, :]
                if c >= 4:
                    rw = cell["rw"][:, 0:nsz]
                else:
                    rw = dfes_rws[ni][:, c, 0:nsz]
                nc.tensor.matmul(box[key][:, 0:nsz], _mm(lhsT, "fc", cfg),
                                 rw, start=(c % 4 == 0),
                                 stop=(c % 4 == 3), skip_group_check=True)
            return (dmaf if c >= 4 else None), mmf

        def df_ih_copy(ni):
            n0, nsz = nsl[ni]
            def f():
                nc.vector.tensor_copy(wbI[:, ni, 0:nsz], box["pwih"][:, 0:nsz])
            return f

        def df_fin(ni):
            n0, nsz = nsl[ni]
            def f():
                nc.vector.tensor_add(wb_sb[:, n0:n0 + nsz], box["pw"][:, 0:nsz],
                                     wbI[:, ni, 0:nsz])
                nc.vector.tensor_add(wb_sb[:, n0:n0 + nsz], wb_sb[:, n0:n0 + nsz],
                                     dfbbc[:, n0:n0 + nsz])
            return f

        def queue_fc_units():
            # All RS-independent work first (ihfc chain + df ih-halves); the
            # es1 ReduceScatter result is only touched ~90 units in, hiding
            # both the collective latency and cross-core launch skew.
            for sp in range(36):
                add_unit(*ihfc_unit(sp))
            add_unit(None, ihfc_fin)
            for c in range(4):
                add_unit(None, transpose_unit(
                    lambda c=c: box["ihsb"][:, c * 128:(c + 1) * 128],
                    lambda c=c: ihT[:, c, :]))
            for ni in range(5):
                for c in range(4, 8):
                    add_unit(*df_unit(ni, c))
                add_unit(None, df_ih_copy(ni))
            noop = lambda: None
            add_unit(es2_dma_all, noop)
            for ni in range(5):
                add_unit(dfes_dma(ni), noop)
            add_unit(None, es1_recv)
            for c in range(8):
                add_unit(None, transpose_unit(
                    lambda c=c: es1sb[:, c * 128:(c + 1) * 128],
                    lambda c=c: es1T[:, c, :]))
            for c in range(8):
                add_unit(*es2_unit(c))
            add_unit(None, es2_fin)
            for c in range(4):
                add_unit(None, transpose_unit(
                    lambda c=c: box["es2sb"][:, c * 128:(c + 1) * 128],
                    lambda c=c: es2T[:, c, :]))
            def L_unit(f, ch):
                def fn():
                    tp = tp_tile([128, S])
                    nc.tensor.transpose(
                        tp, wb_sb[:, f * 256 + ch * 128: f * 256 + (ch + 1) * 128],
                        ident)
                    nc.vector.tensor_copy(L_sb[:, ch, f, :], tp)
                return fn

            def dbias_unit():
                dbp = tp_tile([8, S])
                nc.tensor.transpose(dbp, wb_sb[:, 2048:2056], ident)
                nc.vector.tensor_copy(dbias_t, dbp)
                dma(out=bb, in_=bass.AP(dbias_t.tensor, dbias_t.offset,
                                        [dbias_t.ap[0], [0, 8], dbias_t.ap[1]]))

            for ni in range(5):
                for c in range(4):
                    add_unit(*df_unit(ni, c))
                add_unit(None, df_fin(ni))
                if ni < 4:
                    for f in (2 * ni, 2 * ni + 1):
                        for ch in range(2):
                            add_unit(None, L_unit(f, ch))
                else:
                    add_unit(None, dbias_unit)

        # ---- phase 0: conv1 + ih1 for all samples, es1 units interleaved.
        # Drains split scalar/vector (phase0 was scalar-engine-bound). ----
        ADD = mybir.AluOpType.add
        MAX = mybir.AluOpType.max
        c1os_all = []
        with nc.named_scope("phase0_conv1"):
            for s in range(S):
                c1o = work.tile([128, 2, 484], cat, tag="c1o", bufs=S,
                                name=f"c1o_{s}")
                for c in range(2):
                    p1 = ps.tile([128, 484], F32, tag="mm", bufs=3)
                    nc.tensor.matmul(p1, w1r_t[:, c * 128:(c + 1) * 128],
                                     im1_t[:, s, :], start=True, stop=True)
                    if c == 0:
                        nc.scalar.activation(c1o[:, c, :], p1, RELU,
                                             bias=b1t[:, c:c + 1])
                    else:
                        nc.vector.tensor_scalar(c1o[:, c, :], p1,
                                                b1t[:, c:c + 1], 0.0, ADD, MAX)
                c1os_all.append(c1o)
                pi = ps.tile([64, 144], F32, tag="mm", bufs=3)
                nc.tensor.matmul(pi, ihw1r_t, ihim_t[:, s, :], start=True, stop=True)
                nc.vector.tensor_scalar(pad1[:, s, 1:13, 1:13],
                                        pi.rearrange("p (h w) -> p h w", h=12),
                                        ihb1t, 0.0, ADD, MAX)
                drip(1)

        # ---- phase 2 (early): ih conv2, then drain es1 + trigger the RS ----
        with nc.named_scope("phase2_ih2"):
            for grp in range(2):
                p2i = ps.tile([128, 8, 36], F32, tag="mm", bufs=3)
                sl = slice(grp * 8, (grp + 1) * 8)
                for ky in range(3):
                    for kx in range(3):
                        nc.tensor.matmul(
                            p2i, ihw2t_t[:, ky * 3 + kx, :],
                            _mm(pad1[:, sl, ky:ky + 12:2, kx:kx + 12:2], "conv", cfg),
                            start=(ky == 0 and kx == 0), stop=(ky == 2 and kx == 2))
                nc.scalar.activation(ih2act[:, sl, :], p2i, RELU, bias=ihb2t)
            drip(4)  # last es1 units + fin (ReduceScatter trigger)
            queue_fc_units()
            deferred_const_dmas()

        # ---- phase 1: conv2 + pool per sample; drip between blocks ----
        with nc.named_scope("phase1_conv2"):
            for s in range(S):
                for oc in range(4):
                    p2 = ps.tile([128, 400], F32, tag="mm", bufs=3,
                                 name=f"p2_{s}")
                    for c in range(2):
                        for ky in range(3):
                            for kx in range(3):
                                w = w2t_t[:, oc, c, ky * 3 + kx, :]
                                c1v = c1os_all[s].rearrange(
                                    "p c (h w) -> p c h w", h=22)
                                nc.tensor.matmul(
                                    p2, w,
                                    _mm(c1v[:, c, ky:ky + 20, kx:kx + 20], "conv", cfg),
                                    start=(c == 0 and ky == 0 and kx == 0),
                                    stop=(c == 1 and ky == 2 and kx == 2),
                                    skip_group_check=True)
                    c2o = work.tile([128, 20, 20], cat, tag="c2o")
                    nc.scalar.activation(
                        c2o, p2.rearrange("p (h w) -> p h w", h=20),
                        RELU, bias=b2t[:, oc:oc + 1])
                    tmp = work.tile([128, 10, 20], cat, tag="pooltmp", bufs=1)
                    nc.vector.tensor_max(tmp, c2o[:, 0:20:2, :], c2o[:, 1:20:2, :])
                    nc.vector.tensor_max(
                        pooled[:, oc, s, :].rearrange("p (h w) -> p h w", h=10),
                        tmp[:, :, 0:20:2], tmp[:, :, 1:20:2])
                drip(4)

        # ---- phase 3: conv3, remaining units dripped between ci-chunks ----
        with nc.named_scope("phase3_conv3"):
            pv = pooled.rearrange("p c s (h w) -> p c s h w", h=10)
            ppc = ps.tile([64, 9, S], F32, tag="ppc", bufs=1)
            _P3_TAGS = (("mm", 3), ("mm", 3), ("mm", 3), ("acc0", 1), ("acc1", 1))
            for oc in range(8):
                p3 = []
                for grp in range(2):
                    # 5-slot rotation (3 mm banks + the dead es1 acc banks):
                    # the strided c3a drains take ~2.4us, and with only 3
                    # slots the next oc's matmuls stall on them.
                    tag, nb = _P3_TAGS[(2 * oc + grp) % 5]
                    p3.append(ps.tile([128, 8, 64], F32, tag=tag, bufs=nb,
                                      name=f"p3g{grp}"))
                for c in range(4):
                    w3c = stream.tile([128, 9, 128], cdt, tag="w3", bufs=2)
                    dma(out=w3c, in_=w3t_d.ap()[oc, :, c, :, :])
                    for ky in range(3):
                        for kx in range(3):
                            for grp in range(2):
                                sl = slice(grp * 8, (grp + 1) * 8)
                                nc.tensor.matmul(
                                    p3[grp], w3c[:, ky * 3 + kx, :],
                                    _mm(pv[:, c, sl, ky:ky + 8, kx:kx + 8], "conv", cfg),
                                    start=(c == 0 and ky == 0 and kx == 0),
                                    stop=(c == 3 and ky == 2 and kx == 2),
                                    skip_group_check=True)
                    drip(3, keep=14)
                # c3a stored [128, 64(pos), S]: pc's moving AP then has a
                # contiguous S-run per column group (was single-element
                # stride-2 reads at ~2ns/col, 5x off peak).
                c3a = work.tile([128, 64, S], pat, tag="c3a", bufs=2)
                for grp in range(2):
                    sl = slice(grp * 8, (grp + 1) * 8)
                    dst = bass.AP(c3a.tensor, c3a.offset + grp * 8,
                                  [c3a.ap[0], [1, 8], [S, 64]])
                    if grp == 1:
                        # last drain is tail-critical: split across engines
                        nc.vector.tensor_scalar(dst, p3[grp],
                                                b3t[:, oc:oc + 1], 0.0, ADD, MAX)
                    else:
                        nc.scalar.activation(dst, p3[grp], RELU,
                                             bias=b3t[:, oc:oc + 1])
                pcw_c = stream.tile([128, 9, 64], pdt, tag="pcw", bufs=2)
                dma(out=pcw_c, in_=pcwt_d.ap()[:, oc, :, :])
                drip(2, keep=14)  # cover the c3a activation latency
                for ky in range(3):
                    for kx in range(3):
                        src = bass.AP(
                            c3a.tensor, c3a.offset + (ky * 8 + kx) * S,
                            [c3a.ap[0], [16 * S, 3], [2 * S, 3], [1, S]])
                        nc.tensor.matmul(ppc, pcw_c[:, ky * 3 + kx, :],
                                         _mm(src, "pc", cfg),
                                         start=(oc == 0 and ky == 0 and kx == 0),
                                         stop=(oc == 7 and ky == 2 and kx == 2),
                                         skip_group_check=True)

        # ---- phase 4: pc epilogue + H'' bounce issue (overlaps phase 5) ----
        with nc.named_scope("phase4_pc"):
            pc_act = work.tile([64, 9, S], mybir.dt.bfloat16, tag="pcact", bufs=1)
            nc.scalar.activation(pc_act, ppc, RELU, bias=pcbt)
            # H'' bounce: scr_h [e, kk, g, s].  Single batched scatter/gather
            # (was 8+8 DMA instructions at ~0.8us serial issue each).
            dst = bass.AP(scr_h, 0, [[S, 8], [9 * 8 * S, 8], [8 * S, 9], [1, S]])
            nc.scalar.dma_start(out=dst, in_=pc_act[:, :, :])
            H_t = work.tile([72, 8, S], mybir.dt.bfloat16, tag="Ht", bufs=1)
            src = bass.AP(scr_h, 0, [[9 * 8 * S, 8], [8 * S, 9], [S, 8], [1, S]])
            nc.scalar.dma_start(out=H_t[:, :, :], in_=src)

            drip(2)

        # ---- phase 5: drain remaining units (overlaps H bounce) ----
        with nc.named_scope("phase5_fc"):
            drip(max(0, units_len() - 14))

        # ---- phase 6: dynamic filter tail (all f32) ----
        with nc.named_scope("phase6_tail"):
            drip(7)  # keep the PE fed while the H bounce completes
            # stage 1: t'' [(c,d) 2x128, (g,s) 128]
            tps = ps.tile([128, 2, 8, S], F32, tag="fc", bufs=2)
            for ch in range(2):
                nc.tensor.matmul(tps[:, ch, :, :], bprime_t[:, ch * 128:(ch + 1) * 128],
                                 H_t, start=True, stop=True)
            t_sb = work.tile([128, 2, 8, S], mybir.dt.bfloat16, tag="tsb", bufs=1)
            nc.vector.tensor_copy(t_sb, tps)
            for ch in range(2):
                dst = bass.AP(scr_t, ch * 128 * 8 * S, [[8 * S, 128], [S, 8], [1, S]])
                nc.scalar.dma_start(out=dst, in_=t_sb[:, ch, :, :])
            R_t = work.tile([128, 2, 8, S], mybir.dt.bfloat16, tag="Rt", bufs=1)
            for ch in range(2):
                # R_t[gq*32+c, ch, d, s] = scr_t[(c,d), (ch*4+gq, s)]
                src = bass.AP(scr_t, ch * 4 * S,
                              [[S, 4], [8 * 8 * S, 32], [8 * S, 8], [1, S]])
                nc.scalar.dma_start(out=R_t[:, ch, :, :], in_=src)
            drip(units_len())  # L transposes etc. run during the R bounce
            # stage 2: out2 [8(f), 8(d), S]
            o2 = ps.tile([8, 8, S], F32, tag="fc", bufs=2)
            for s in range(S):
                for ch in range(2):
                    nc.tensor.matmul(o2[:, :, s], L_sb[:, ch, :, s], R_t[:, ch, :, s],
                                     start=(ch == 0), stop=(ch == 1))
            feat = work.tile([8, 8, S], mybir.dt.bfloat16, tag="feat", bufs=1)
            nc.vector.tensor_add(feat, o2, bb)
            nc.vector.tensor_relu(feat, feat)
            xps = ps.tile([S, 16], F32, tag="fc", bufs=2)
            for d in range(8):
                nc.tensor.matmul(xps, feat[:, d, :], c2wt_t[:, d, :],
                                 start=(d == 0), stop=(d == 7))
            x_sb = work.tile([S, 16], F32, tag="xsb", bufs=1)
            nc.vector.tensor_add(x_sb, xps, c2bbc)
            dma(out=out_d.ap(), in_=x_sb)

    nc.compile()
    return nc


def _prep_inputs(inputs, cfg):
    """Host-side: shard activations, rearrange weights into tile layouts."""
    i = {k: np.asarray(v, dtype=np.float32) for k, v in inputs.items()}
    cnp = _DT_NP[cfg["conv"]]
    fnp = _DT_NP[cfg["fc"]]
    pnp = _DT_NP[cfg["pc"]]

    obs = i["obs_encoding"].reshape(B, 16384)
    image = i["patch"][:, 0]   # [B,24,24]
    inhand = i["patch"][:, 1]  # [B,24,24]

    # conv1 im2col: [B, 9, 484]
    sw = np.lib.stride_tricks.sliding_window_view(image, (3, 3), axis=(1, 2))
    im1 = sw.transpose(0, 3, 4, 1, 2).reshape(B, 9, 484)
    # ih conv1 im2col (stride 2, pad 1): [B, 9, 144]
    ip = np.pad(inhand, ((0, 0), (1, 1), (1, 1)))
    swi = np.lib.stride_tricks.sliding_window_view(ip, (3, 3), axis=(1, 2))[:, ::2, ::2]
    ihim = swi.transpose(0, 3, 4, 1, 2).reshape(B, 9, 144)

    def conv_w_t(w, nchunk):  # [O, I, 3, 3] -> [128, nchunk, 9, O]
        O, I = w.shape[0], w.shape[1]
        return np.ascontiguousarray(
            w.reshape(O, nchunk, 128, 9).transpose(2, 1, 3, 0))

    # conv2 weights -> [ci_i(128), oc_chunk(4), ci_chunk(2), 9, oc_i(128)]
    w2 = i["enc_w2"].reshape(4, 128, 2, 128, 9)
    w2t = np.ascontiguousarray(w2.transpose(3, 0, 2, 4, 1))

    obsT_full = np.ascontiguousarray(obs.T)  # [16384, 128]

    shared = {
        "w1r": np.ascontiguousarray(i["enc_w1"].reshape(256, 9).T).astype(cnp),
        "w2t": w2t.astype(cnp),
        "w3t": np.ascontiguousarray(
            i["enc_w3"].reshape(8, 128, 4, 128, 9).transpose(0, 3, 2, 4, 1)).astype(cnp),
        "pcwt": conv_w_t(i["pc_w"], 8).astype(pnp),
        "ihw1r": np.ascontiguousarray(i["ih_w1"].reshape(64, 9).T).astype(cnp),
        "ihw2t": np.ascontiguousarray(
            i["ih_w2"].reshape(128, 64, 9).transpose(1, 2, 0)).astype(cnp),
        # feature order f' = sp*128 + ci  (ih flatten is ci*36 + sp)
        "ihfcwt": np.ascontiguousarray(
            i["ih_fc_w"].reshape(512, 128, 36).transpose(2, 1, 0).reshape(4608, 512)
        ).astype(fnp),
        "esw2t": np.ascontiguousarray(i["es_w2"].T).astype(fnp),
        "dfwt": np.ascontiguousarray(i["df_w"].T).astype(fnp),
        # basis [c,d,e,kh,kw] -> [(e,kk), (c,d)]
        "bprime": np.ascontiguousarray(
            i["basis"].reshape(32, 8, 8, 9).transpose(2, 3, 0, 1).reshape(72, 256)
        ).astype(ml_dtypes.bfloat16),
        # c2_w [o, f*8+d] -> [f, d, o]
        "c2wt": np.ascontiguousarray(
            i["c2_w"].reshape(16, 8, 8).transpose(1, 2, 0)).astype(ml_dtypes.bfloat16),
    }
    # conv biases packed per-partition: [b1(2) b2(4) b3(8) ihb2(1) pcb(1) ihb1(1)]
    blobA = np.zeros((128, 17), np.float32)
    blobA[:, 0:2] = i["enc_b1"].reshape(2, 128).T
    blobA[:, 2:6] = i["enc_b2"].reshape(4, 128).T
    blobA[:, 6:14] = i["enc_b3"].reshape(8, 128).T
    blobA[:, 14] = i["ih_b2"]
    blobA[0:64, 15] = i["pc_b"]
    blobA[0:64, 16] = i["ih_b1"]
    shared["bblobA"] = blobA
    # S-broadcast FC biases: [esb1(1024) esb2(512) ihfcb(512) c2b(16) dfb(2056)]
    blobB = np.concatenate([i["es_b1"], i["es_b2"], i["ih_fc_b"], i["c2_b"],
                            i["df_b"]]).astype(np.float32)
    shared["bblobB"] = np.ascontiguousarray(np.broadcast_to(blobB, (S, 4120)))
    esw1_full = np.ascontiguousarray(i["es_w1"].T)  # [16384, 1024]
    in_maps = []
    for c in range(NCORES):
        sl = slice(c * S, (c + 1) * S)
        ksl = slice(c * KSH, (c + 1) * KSH)
        m = dict(shared)
        # es1 K-shard: obs rows for ALL samples, [p, kc, s] contiguous layout
        m["obsT"] = np.ascontiguousarray(
            obsT_full[ksl].reshape(KSH // 128, 128, B).transpose(1, 0, 2)).astype(fnp)
        m["esw1t"] = np.ascontiguousarray(esw1_full[ksl]).astype(fnp)
        m["im1"] = np.ascontiguousarray(im1[sl].transpose(1, 0, 2)).astype(cnp)
        m["ihim"] = np.ascontiguousarray(ihim[sl].transpose(1, 0, 2)).astype(cnp)
        in_maps.append(m)
    return in_maps


_CACHE = {}


def _get_nc(cfg):
    key = tuple(sorted(cfg.items()))
    if key not in _CACHE:
        _CACHE[key] = build(cfg)
    return _CACHE[key]


def run(inputs, cfg=None, trace=False):
    cfg = cfg or DEFAULT_CFG
    nc = _get_nc(cfg)
    in_maps = _prep_inputs(inputs, cfg)
    res = run_bass_kernel_spmd(nc, in_maps, list(range(NCORES)), trace=trace)
    out = np.concatenate([res.results[c]["out"] for c in range(NCORES)], axis=0)
    return out.astype(np.float32), res


def kernel(**inputs) -> np.ndarray:
    out, _ = run(inputs)
    return out



# revision 35
# speedup vs baseline: 1.0592x; 1.0592x over previous
"""Trainium2 Bass kernel for nn_EquShiftQ2DF (dense_cnn).

Data-parallel over 8 NeuronCores: each core processes 16 of the 128 samples
for the conv/ih/df paths; all conv weights are replicated (host pre-arranges
each weight into the exact SBUF tile layout so every DMA is a contiguous 2D
copy).

The big es FC1 (16384->1024, 67MB of weights) is K-SHARDED across the 8
cores: core c takes K-rows [c*2048, (c+1)*2048) of es_w1 for ALL 128 samples,
computes a partial es1 [128, 1024] in PSUM, and a ReduceScatter(add) over the
sample dim hands each core its own 16 samples' finished es1 [16, 1024].  This
cuts the per-core es1 weight stream 8x and the PE row count 8x.

SKEW ROBUSTNESS: the 8 cores launch with 20-100us of random skew (measured),
so the RS completes at an unpredictable wall time.  Everything that consumes
the RS result (es1 bias/relu, es1T transposes, es2 chain, the es-half of the
df contraction) is pushed as late as possible: the df contraction is split
into its ih-half (RS-independent, runs early) and es-half (late), and the RS
receive is issued under tc.tile_wait_until(0.25) so the Tile scheduler places
it late in every engine's program instead of head-of-line-blocking the sync
DMA queue / vector queue while the collective drains.

Per-core pipeline (S=16 samples):
  es branch   : K-sharded FC1 (above) -> FC(1024->512) sample-local.
  ih branch   : conv s2 x2 (as 9-offset shifted matmuls) + FC(4608->512).
  enc convs   : conv1 (host im2col, K=9), conv2/conv3 as 9-offset shifted
                matmuls with K=ci chunks accumulated in PSUM, 2x2 maxpool on
                DVE, pc conv (stride 2; c3a stored position-major so pc's
                moving AP has contiguous 16-sample runs).
  hypernet    : df FC -> per-sample filter coeffs; dynamic-filter contraction
                via a fixed-basis matmul (t = basis^T @ H) plus per-sample
                [128x8]x[128x8] matmuls (bf16); layout shuffles via two DRAM
                bounces batched into 1-2 DMA instructions each.

Engine balance: conv drains are split scalar/vector; conv3's PSUM rotates
through 5 banks (3 'mm' + the dead es1 accumulator banks) so the ~2.4us
strided drains never stall the next oc; RS-dependent SBUF-only epilogues run
on the otherwise-idle gpsimd engine.

Per-stage dtype is configurable: 'bf16' | 'f32r' | 'f32'.  bf16 halves the
HBM weight stream AND halves the PE stationary-load (LDWEIGHTS) time; the
dynamic-filter tail (basis/H/t/L/R/c2) also runs bf16 (adds ~1.3e-3 rel err,
well under the 2e-2 gate).
"""
import numpy as np
import ml_dtypes
from contextlib import ExitStack

import concourse.bass as bass
import concourse.tile as tile
from concourse import bacc, mybir
from concourse.bass_utils import run_bass_kernel_spmd
from concourse.masks import make_identity

NCORES = 8
B = 128
S = B // NCORES  # samples per core
KSH = 16384 // NCORES  # es1 K-shard rows per core

DEFAULT_CFG = {"conv": "bf16", "fc": "bf16", "pc": "bf16"}

_DT_DRAM = {"bf16": mybir.dt.bfloat16, "f32r": mybir.dt.float32r, "f32": mybir.dt.float32}
_DT_ACT = {"bf16": mybir.dt.bfloat16, "f32r": mybir.dt.float32r, "f32": mybir.dt.float32}
_DT_NP = {"bf16": ml_dtypes.bfloat16, "f32r": np.float32, "f32": np.float32}

F32 = mybir.dt.float32


def _mm(ap, key, cfg):
    """Cast an activation AP to the stage's matmul dtype."""
    if cfg[key] == "f32r" and ap.dtype == F32:
        return ap.bitcast(mybir.dt.float32r)
    return ap


def build(cfg):
    nc = bacc.Bacc("TRN2", target_bir_lowering=False, debug=False, num_devices=NCORES)
    cdt = _DT_DRAM[cfg["conv"]]
    fdt = _DT_DRAM[cfg["fc"]]
    pdt = _DT_DRAM[cfg["pc"]]
    cat = _DT_ACT[cfg["conv"]]
    fat = _DT_ACT[cfg["fc"]]
    pat = _DT_ACT[cfg["pc"]]

    D = nc.dram_tensor
    # per-core activations
    obsT_d = D("obsT", [128, KSH // 128, B], fdt, kind="ExternalInput")  # [p, kc, s]
    im1_d = D("im1", [9, S, 484], cdt, kind="ExternalInput")
    ihim_d = D("ihim", [9, S, 144], cdt, kind="ExternalInput")
    # replicated weights (host pre-arranged to tile layouts)
    w1r_d = D("w1r", [9, 256], cdt, kind="ExternalInput")
    w2t_d = D("w2t", [128, 4, 2, 9, 128], cdt, kind="ExternalInput")
    w3t_d = D("w3t", [8, 128, 4, 9, 128], cdt, kind="ExternalInput")
    pcwt_d = D("pcwt", [128, 8, 9, 64], pdt, kind="ExternalInput")
    ihw1r_d = D("ihw1r", [9, 64], cdt, kind="ExternalInput")
    ihw2t_d = D("ihw2t", [64, 9, 128], cdt, kind="ExternalInput")
    ihfcwt_d = D("ihfcwt", [4608, 512], fdt, kind="ExternalInput")
    esw1t_d = D("esw1t", [KSH, 1024], fdt, kind="ExternalInput")  # per-core K-shard
    esw2t_d = D("esw2t", [1024, 512], fdt, kind="ExternalInput")
    dfwt_d = D("dfwt", [1024, 2056], fdt, kind="ExternalInput")
    bprime_d = D("bprime", [72, 256], mybir.dt.bfloat16, kind="ExternalInput")
    c2wt_d = D("c2wt", [8, 8, 16], mybir.dt.bfloat16, kind="ExternalInput")
    # all conv biases packed per-partition: [b1(2) b2(4) b3(8) ihb2(1) pcb(1) ihb1(1)]
    bblobA_d = D("bblobA", [128, 17], F32, kind="ExternalInput")
    # all S-broadcast FC biases: [esb1(1024) esb2(512) ihfcb(512) c2b(16) dfb(2056)]
    bblobB_d = D("bblobB", [S, 4120], F32, kind="ExternalInput")
    out_d = D("out", [S, 2, 8], F32, kind="ExternalOutput")
    scr_h = D("scr_h", [8, 9, 8, S], mybir.dt.bfloat16)   # [e, kk, g, s]
    scr_t = D("scr_t", [256, 8 * S], mybir.dt.bfloat16)   # [(c,d), (g,s)]
    CCDT = mybir.dt.bfloat16  # RS payload dtype (halves fabric bytes)
    cc_in = D("es1cc_in", [B, 1024], CCDT)   # partial es1
    cc_out = D("es1cc_out", [S, 1024], CCDT)  # post-RS

    RELU = mybir.ActivationFunctionType.Relu

    with tile.TileContext(nc) as tc, ExitStack() as ctx:
        wts = ctx.enter_context(tc.tile_pool(name="wts", bufs=1))
        stream = ctx.enter_context(tc.tile_pool(name="stream", bufs=3))
        acts = ctx.enter_context(tc.tile_pool(name="acts", bufs=1))
        work = ctx.enter_context(tc.tile_pool(name="work", bufs=2))
        ps = ctx.enter_context(tc.tile_pool(name="ps", bufs=1, space="PSUM"))

        dma = nc.sync.dma_start

        # ---- phase 0: constants / weights; issue order = DMA queue order,
        # so first-needed tensors go first (conv1+ih1 inputs, then conv2) ----
        w1r_t = wts.tile([9, 256], cdt)
        dma(out=w1r_t, in_=w1r_d.ap())
        # conv1 input immediately behind its weights: first matmul at ~3us
        im1_t = wts.tile([9, S, 484], cdt)
        dma(out=im1_t[:, 0:1, :], in_=im1_d.ap()[:, 0:1, :])
        ihw1r_t = wts.tile([9, 64], cdt)
        dma(out=ihw1r_t, in_=ihw1r_d.ap())
        # conv biases, one packed DMA
        bblobA = wts.tile([128, 17], F32)
        dma(out=bblobA, in_=bblobA_d.ap())
        b1t = bblobA[:, 0:2]
        b2t = bblobA[:, 2:6]
        b3t = bblobA[:, 6:14]
        ihb2t = bblobA[:, 14:15]
        pcbt = bblobA[0:64, 15:16]
        ihb1t = bblobA[0:64, 16:17]
        # whole im2col inputs (small): no per-sample DMAs in phase 1
        ihim_t = wts.tile([9, S, 144], cdt)
        dma(out=ihim_t, in_=ihim_d.ap())
        dma(out=im1_t[:, 1:, :], in_=im1_d.ap()[:, 1:, :])
        obsT_t = wts.tile([128, KSH // 128, B], fdt)
        dma(out=obsT_t, in_=obsT_d.ap())
        ihw2t_t = wts.tile([64, 9, 128], cdt)
        dma(out=ihw2t_t, in_=ihw2t_d.ap())
        # conv2 weights: oc0 up-front (needed at phase-1 start); oc1-3 plus
        # cold constants are issued after phase 0 so the es1 stream isn't
        # stuck behind them in the DMA queue.
        w2t_t = wts.tile([128, 4, 2, 9, 128], cdt)
        dma(out=w2t_t[:, 0, :, :, :], in_=w2t_d.ap()[:, 0, :, :, :])
        bblobB = wts.tile([S, 4120], F32)
        dma(out=bblobB, in_=bblobB_d.ap())
        esb1bc = bblobB[:, 0:1024]
        esb2bc = bblobB[:, 1024:1536]
        ihfcbc = bblobB[:, 1536:2048]
        c2bbc = bblobB[:, 2048:2064]
        dfbbc = bblobB[:, 2064:4120]
        bprime_t = wts.tile([72, 256], mybir.dt.bfloat16)
        c2wt_t = wts.tile([8, 8, 16], mybir.dt.bfloat16)
        ident = wts.tile([16, 16], F32)
        make_identity(nc, ident)

        def deferred_const_dmas():
            for oc in range(1, 4):
                dma(out=w2t_t[:, oc, :, :, :], in_=w2t_d.ap()[:, oc, :, :, :])
            dma(out=bprime_t, in_=bprime_d.ap())
            dma(out=c2wt_t, in_=c2wt_d.ap())

        # persistent activations
        pooled = acts.tile([128, 4, S, 100], cat)
        pad1 = acts.tile([64, S, 14, 14], cat)
        nc.vector.memset(pad1.bitcast(F32) if pad1.dtype == mybir.dt.float32r else pad1, 0.0)
        ih2act = acts.tile([128, S, 36], fat)
        es1sb = acts.tile([S, 1024], F32)
        es1T = acts.tile([128, 8, S], fat)
        es2T = acts.tile([128, 4, S], fat)
        ihT = acts.tile([128, 4, S], fat)
        wb_sb = acts.tile([S, 2056], F32)
        L_sb = acts.tile([128, 2, 8, S], mybir.dt.bfloat16)
        dbias_t = acts.tile([8, S], F32)
        bb = acts.tile([8, 8, S], F32)

        es1_ps = []
        for h in range(2):
            es1_ps.append(ps.tile([B, 512], F32, tag=f"acc{h}", bufs=1, name=f"es1ps{h}"))

        # ---------- drip-feed unit queue: DMA-heavy FC work interleaved ----------
        # Units are (dma_fn, mm_fn) pairs; the dma side runs LOOK units ahead
        # of the matmul side so a dripped matmul never waits on its own DMA.
        from collections import deque
        dma_q = deque()
        mm_q = deque()
        LOOK = 2
        dstate = {"d": 0, "m": 0}

        def add_unit(dma_fn, mm_fn):
            dma_q.append(dma_fn)
            mm_q.append(mm_fn)

        def drip(n, keep=0):
            for _ in range(n):
                if len(mm_q) <= keep:
                    return
                while dma_q and dstate["d"] < dstate["m"] + 1 + LOOK:
                    fn = dma_q.popleft()
                    dstate["d"] += 1
                    if fn:
                        fn()
                f = mm_q.popleft()
                dstate["m"] += 1
                f()

        def units_len():
            return len(mm_q)

        NK2 = KSH // 256  # es1 double-K-chunks per core (8)

        def es1_unit(kc2):
            # two 128-row K-chunks per unit: halves the sync-queue DMA count
            # so the es1 weight stream lands early enough to trigger the
            # ReduceScatter by ~35us instead of ~70us.
            cell = {}
            def dmaf():
                rw = stream.tile([128, 2, 1024], fdt, tag="esw1", bufs=3,
                                 name="esw1t")
                src = esw1t_d.ap()[kc2 * 256:(kc2 + 1) * 256, :]
                dma(out=rw, in_=src.rearrange("(c p) n -> p c n", c=2))
                cell["rw"] = rw
            def mmf():
                rw = cell["rw"]
                for c in range(2):
                    kc = 2 * kc2 + c
                    for h in range(2):
                        nc.tensor.matmul(es1_ps[h], _mm(obsT_t[:, kc, :], "fc", cfg),
                                         rw[:, c, h * 512:(h + 1) * 512],
                                         start=(kc == 0), stop=(kc == 2 * NK2 - 1),
                                         skip_group_check=True)
            return dmaf, mmf

        for kc2 in range(NK2):
            add_unit(*es1_unit(kc2))

        box = {}

        def es1_fin():
            # partial [128, 1024] -> DRAM -> ReduceScatter(add, sample dim).
            # Trigger only: the receive DMA + bias/relu live in es1_recv, queued
            # ~40 units later so the RS-gated DMA never heads the sync queue
            # (head-of-line there stalled the whole FC weight stream ~37us).
            part = work.tile([B, 1024], CCDT, tag="es1part", bufs=1, name="es1part")
            for h in range(2):
                nc.vector.tensor_copy(part[:, h * 512:(h + 1) * 512], es1_ps[h])
            dma(out=cc_in.ap(), in_=part)
            nc.gpsimd.collective_compute(
                "ReduceScatter", mybir.AluOpType.add,
                replica_groups=[list(range(NCORES))],
                ins=[cc_in.ap().opt()], outs=[cc_out.ap().opt()])

        def es1_recv():
            # tile_wait_until: the scheduler otherwise places this DMA early
            # in the sync program (its model has zero cross-core skew), and a
            # late-arriving ReduceScatter then head-of-line-blocks the whole
            # weight stream.  Forcing sim-readiness to 250us pushes it (and
            # the dependent es chain) late in every engine's program.
            with tc.tile_wait_until(0.25):
                pre = work.tile([S, 1024], CCDT, tag="es1pre", bufs=1, name="es1pre")
                dma(out=pre, in_=cc_out.ap())
                nc.vector.tensor_add(es1sb, pre, esb1bc)
                nc.vector.tensor_relu(es1sb, es1sb)

        add_unit(None, es1_fin)

        # Transposes ping-pong through the (dead-by-then) es1 PSUM banks so
        # they don't fight the fc accumulators for bufs (was serializing each
        # transpose behind the previous vector copy, ~322ns apiece).
        tp_state = {"i": 0}

        def tp_tile(shape):
            # fc-only ping-pong; acc0/acc1 are lent to conv3's p3 rotation
            return ps.tile(shape, F32, tag="fc", bufs=2, name="tpos")

        def transpose_unit(src_fn, dst_fn):
            def f():
                tp = tp_tile([128, S])
                nc.tensor.transpose(tp, src_fn(), ident)
                nc.vector.tensor_copy(dst_fn(), tp)
            return f

        # es2/df-es weight streams are prefetched by dma-only units queued
        # BEFORE es1_recv: the recv carries tile_wait_until(0.25), and any DMA
        # with a higher build priority would otherwise be scheduled after it
        # on the sync queue, starving the late es chain (~13us PE stall).
        es2_rws = []

        def es2_dma_all():
            # 2 batched DMAs (was 8): the prefetch burst was congesting the
            # sync queue (~0.6us serial issue each) right when phase1's
            # on-demand weight stream needed it.
            for half in range(2):
                rw = stream.tile([128, 4, 512], fdt, tag="esw2", bufs=2,
                                 name="esw2t")
                s4 = esw2t_d.ap()[half * 512:(half + 1) * 512, :]
                dma(out=rw, in_=s4.rearrange("(c p) n -> p c n", c=4))
                es2_rws.append(rw)

        def es2_unit(c):
            def mmf():
                if c == 0:
                    box["p_es2"] = ps.tile([S, 512], F32, tag="fc", bufs=2,
                                           name="pes2")
                nc.tensor.matmul(box["p_es2"], _mm(es1T[:, c, :], "fc", cfg),
                                 es2_rws[c // 4][:, c % 4, :],
                                 start=(c == 0), stop=(c == 7),
                                 skip_group_check=True)
            return None, mmf

        def es2_fin():
            es2sb = work.tile([S, 512], F32, tag="fcout")
            nc.vector.tensor_add(es2sb, box["p_es2"], esb2bc)
            nc.vector.tensor_relu(es2sb, es2sb)
            box["es2sb"] = es2sb

        def ihfc_unit(sp):
            cell = {}
            def dmaf():
                rw = stream.tile([128, 512], fdt, tag="ihfcw", name="ihfcwt")
                dma(out=rw, in_=ihfcwt_d.ap()[sp * 128:(sp + 1) * 128, :])
                cell["rw"] = rw
            def mmf():
                if sp == 0:
                    box["p_ihfc"] = ps.tile([S, 512], F32, tag="fc", bufs=2,
                                            name="pihfc")
                nc.tensor.matmul(box["p_ihfc"], _mm(ih2act[:, :, sp], "fc", cfg),
                                 cell["rw"], start=(sp == 0), stop=(sp == 35),
                                 skip_group_check=True)
            return dmaf, mmf

        def ihfc_fin():
            ihsb = work.tile([S, 512], F32, tag="fcout")
            nc.vector.tensor_add(ihsb, box["p_ihfc"], ihfcbc)
            nc.vector.tensor_relu(ihsb, ihsb)
            box["ihsb"] = ihsb

        nsl = [(0, 512), (512, 512), (1024, 512), (1536, 512), (2048, 8)]
        # df contraction split: ih half (c=4..7, RS-independent) runs early,
        # es half (c=0..3) late.  This pushes every RS-dependent unit as late
        # as possible so cross-core launch skew (measured 20-60us) is absorbed
        # instead of stalling the PE at the ReduceScatter.
        wbI = acts.tile([S, 5, 512], F32)

        dfes_rws = {}

        def dfes_dma(ni):
            n0, nsz = nsl[ni]
            def dmaf():
                rw = stream.tile([128, 4, 512], fdt, tag="dfwes", bufs=5,
                                 name="dfwes_t")
                s4 = dfwt_d.ap()[0:512, n0:n0 + nsz]
                dma(out=rw[:, :, 0:nsz],
                    in_=s4.rearrange("(c p) n -> p c n", c=4))
                dfes_rws[ni] = rw
            return dmaf

        def df_unit(ni, c):
            n0, nsz = nsl[ni]
            key = "pwih" if c >= 4 else "pw"
            cell = {}
            def dmaf():
                rw = stream.tile([128, 512], fdt, tag="dfw", name="dfwt_t")
                dma(out=rw[:, 0:nsz], in_=dfwt_d.ap()[c * 128:(c + 1) * 128, n0:n0 + nsz])
                cell["rw"] = rw
            def mmf():
                if c % 4 == 0:
                    box[key] = ps.tile([S, 512], F32, tag="fc", bufs=2, name=key)
                lhsT = es2T[:, c, :] if c < 4 else ihT[:, c - 4, :]
                if c >= 4:
                    rw = cell["rw"][:, 0:nsz]
                else:
                    rw = dfes_rws[ni][:, c, 0:nsz]
                nc.tensor.matmul(box[key][:, 0:nsz], _mm(lhsT, "fc", cfg),
                                 rw, start=(c % 4 == 0),
                                 stop=(c % 4 == 3), skip_group_check=True)
            return (dmaf if c >= 4 else None), mmf

        def df_ih_copy(ni):
            n0, nsz = nsl[ni]
            def f():
                nc.vector.tensor_copy(wbI[:, ni, 0:nsz], box["pwih"][:, 0:nsz])
            return f

        def df_fin(ni):
            n0, nsz = nsl[ni]
            def f():
                nc.vector.tensor_add(wb_sb[:, n0:n0 + nsz], box["pw"][:, 0:nsz],
                                     wbI[:, ni, 0:nsz])
                nc.vector.tensor_add(wb_sb[:, n0:n0 + nsz], wb_sb[:, n0:n0 + nsz],
                                     dfbbc[:, n0:n0 + nsz])
            return f

        def queue_fc_units():
            # All RS-independent work first (ihfc chain + df ih-halves); the
            # es1 ReduceScatter result is only touched ~90 units in, hiding
            # both the collective latency and cross-core launch skew.
            for sp in range(36):
                add_unit(*ihfc_unit(sp))
            add_unit(None, ihfc_fin)
            for c in range(4):
                add_unit(None, transpose_unit(
                    lambda c=c: box["ihsb"][:, c * 128:(c + 1) * 128],
                    lambda c=c: ihT[:, c, :]))
            for ni in range(5):
                for c in range(4, 8):
                    add_unit(*df_unit(ni, c))
                add_unit(None, df_ih_copy(ni))
            noop = lambda: None
            add_unit(es2_dma_all, noop)
            for ni in range(5):
                add_unit(dfes_dma(ni), noop)
            add_unit(None, es1_recv)
            for c in range(8):
                add_unit(None, transpose_unit(
                    lambda c=c: es1sb[:, c * 128:(c + 1) * 128],
                    lambda c=c: es1T[:, c, :]))
            for c in range(8):
                add_unit(*es2_unit(c))
            add_unit(None, es2_fin)
            for c in range(4):
                add_unit(None, transpose_unit(
                    lambda c=c: box["es2sb"][:, c * 128:(c + 1) * 128],
                    lambda c=c: es2T[:, c, :]))
            def L_unit(f, ch):
                def fn():
                    tp = tp_tile([128, S])
                    nc.tensor.transpose(
                        tp, wb_sb[:, f * 256 + ch * 128: f * 256 + (ch + 1) * 128],
                        ident)
                    nc.vector.tensor_copy(L_sb[:, ch, f, :], tp)
                return fn

            def dbias_unit():
                dbp = tp_tile([8, S])
                nc.tensor.transpose(dbp, wb_sb[:, 2048:2056], ident)
                nc.vector.tensor_copy(dbias_t, dbp)
                dma(out=bb, in_=bass.AP(dbias_t.tensor, dbias_t.offset,
                                        [dbias_t.ap[0], [0, 8], dbias_t.ap[1]]))

            for ni in range(5):
                for c in range(4):
                    add_unit(*df_unit(ni, c))
                add_unit(None, df_fin(ni))
                if ni < 4:
                    for f in (2 * ni, 2 * ni + 1):
                        for ch in range(2):
                            add_unit(None, L_unit(f, ch))
                else:
                    add_unit(None, dbias_unit)

        # ---- phase 0: conv1 + ih1 for all samples, es1 units interleaved.
        # Drains split scalar/vector (phase0 was scalar-engine-bound). ----
        ADD = mybir.AluOpType.add
        MAX = mybir.AluOpType.max
        c1os_all = []
        with nc.named_scope("phase0_conv1"):
            for s in range(S):
                c1o = work.tile([128, 2, 484], cat, tag="c1o", bufs=S,
                                name=f"c1o_{s}")
                for c in range(2):
                    p1 = ps.tile([128, 484], F32, tag="mm", bufs=3)
                    nc.tensor.matmul(p1, w1r_t[:, c * 128:(c + 1) * 128],
                                     im1_t[:, s, :], start=True, stop=True)
                    if c == 0:
                        nc.scalar.activation(c1o[:, c, :], p1, RELU,
                                             bias=b1t[:, c:c + 1])
                    else:
                        nc.vector.tensor_scalar(c1o[:, c, :], p1,
                                                b1t[:, c:c + 1], 0.0, ADD, MAX)
                c1os_all.append(c1o)
                pi = ps.tile([64, 144], F32, tag="mm", bufs=3)
                nc.tensor.matmul(pi, ihw1r_t, ihim_t[:, s, :], start=True, stop=True)
                nc.vector.tensor_scalar(pad1[:, s, 1:13, 1:13],
                                        pi.rearrange("p (h w) -> p h w", h=12),
                                        ihb1t, 0.0, ADD, MAX)
                drip(1)

        # ---- phase 2 (early): ih conv2, then drain es1 + trigger the RS ----
        with nc.named_scope("phase2_ih2"):
            for grp in range(2):
                p2i = ps.tile([128, 8, 36], F32, tag="mm", bufs=3)
                sl = slice(grp * 8, (grp + 1) * 8)
                for ky in range(3):
                    for kx in range(3):
                        nc.tensor.matmul(
                            p2i, ihw2t_t[:, ky * 3 + kx, :],
                            _mm(pad1[:, sl, ky:ky + 12:2, kx:kx + 12:2], "conv", cfg),
                            start=(ky == 0 and kx == 0), stop=(ky == 2 and kx == 2))
                nc.scalar.activation(ih2act[:, sl, :], p2i, RELU, bias=ihb2t)
            drip(4)  # last es1 units + fin (ReduceScatter trigger)
            queue_fc_units()
            deferred_const_dmas()

        # ---- phase 1: conv2 + pool per sample; drip between blocks ----
        with nc.named_scope("phase1_conv2"):
            for s in range(S):
                for oc in range(4):
                    p2 = ps.tile([128, 400], F32, tag="mm", bufs=3,
                                 name=f"p2_{s}")
                    for c in range(2):
                        for ky in range(3):
                            for kx in range(3):
                                w = w2t_t[:, oc, c, ky * 3 + kx, :]
                                c1v = c1os_all[s].rearrange(
                                    "p c (h w) -> p c h w", h=22)
                                nc.tensor.matmul(
                                    p2, w,
                                    _mm(c1v[:, c, ky:ky + 20, kx:kx + 20], "conv", cfg),
                                    start=(c == 0 and ky == 0 and kx == 0),
                                    stop=(c == 1 and ky == 2 and kx == 2),
                                    skip_group_check=True)
                    c2o = work.tile([128, 20, 20], cat, tag="c2o")
                    nc.scalar.activation(
                        c2o, p2.rearrange("p (h w) -> p h w", h=20),
                        RELU, bias=b2t[:, oc:oc + 1])
                    tmp = work.tile([128, 10, 20], cat, tag="pooltmp", bufs=1)
                    nc.vector.tensor_max(tmp, c2o[:, 0:20:2, :], c2o[:, 1:20:2, :])
                    nc.vector.tensor_max(
                        pooled[:, oc, s, :].rearrange("p (h w) -> p h w", h=10),
                        tmp[:, :, 0:20:2], tmp[:, :, 1:20:2])
                drip(4)

        # ---- phase 3: conv3, remaining units dripped between ci-chunks ----
        with nc.named_scope("phase3_conv3"):
            pv = pooled.rearrange("p c s (h w) -> p c s h w", h=10)
            ppc = ps.tile([64, 9, S], F32, tag="ppc", bufs=1)
            _P3_TAGS = (("mm", 3), ("mm", 3), ("mm", 3), ("acc0", 1), ("acc1", 1))
            for oc in range(8):
                p3 = []
                for grp in range(2):
                    # 5-slot rotation (3 mm banks + the dead es1 acc banks):
                    # the strided c3a drains take ~2.4us, and with only 3
                    # slots the next oc's matmuls stall on them.
                    tag, nb = _P3_TAGS[(2 * oc + grp) % 5]
                    p3.append(ps.tile([128, 8, 64], F32, tag=tag, bufs=nb,
                                      name=f"p3g{grp}"))
                for c in range(4):
                    w3c = stream.tile([128, 9, 128], cdt, tag="w3", bufs=2)
                    dma(out=w3c, in_=w3t_d.ap()[oc, :, c, :, :])
                    for ky in range(3):
                        for kx in range(3):
                            for grp in range(2):
                                sl = slice(grp * 8, (grp + 1) * 8)
                                nc.tensor.matmul(
                                    p3[grp], w3c[:, ky * 3 + kx, :],
                                    _mm(pv[:, c, sl, ky:ky + 8, kx:kx + 8], "conv", cfg),
                                    start=(c == 0 and ky == 0 and kx == 0),
                                    stop=(c == 3 and ky == 2 and kx == 2),
                                    skip_group_check=True)
                    drip(3, keep=20)
                # c3a stored [128, 64(pos), S]: pc's moving AP then has a
                # contiguous S-run per column group (was single-element
                # stride-2 reads at ~2ns/col, 5x off peak).
                c3a = work.tile([128, 64, S], pat, tag="c3a", bufs=2)
                for grp in range(2):
                    sl = slice(grp * 8, (grp + 1) * 8)
                    dst = bass.AP(c3a.tensor, c3a.offset + grp * 8,
                                  [c3a.ap[0], [1, 8], [S, 64]])
                    if grp == 1:
                        # last drain is tail-critical: split across engines
                        nc.vector.tensor_scalar(dst, p3[grp],
                                                b3t[:, oc:oc + 1], 0.0, ADD, MAX)
                    else:
                        nc.scalar.activation(dst, p3[grp], RELU,
                                             bias=b3t[:, oc:oc + 1])
                pcw_c = stream.tile([128, 9, 64], pdt, tag="pcw", bufs=2)
                dma(out=pcw_c, in_=pcwt_d.ap()[:, oc, :, :])
                drip(2, keep=20)  # cover the c3a activation latency
                for ky in range(3):
                    for kx in range(3):
                        src = bass.AP(
                            c3a.tensor, c3a.offset + (ky * 8 + kx) * S,
                            [c3a.ap[0], [16 * S, 3], [2 * S, 3], [1, S]])
                        nc.tensor.matmul(ppc, pcw_c[:, ky * 3 + kx, :],
                                         _mm(src, "pc", cfg),
                                         start=(oc == 0 and ky == 0 and kx == 0),
                                         stop=(oc == 7 and ky == 2 and kx == 2),
                                         skip_group_check=True)

        # ---- phase 4: pc epilogue + H'' bounce issue (overlaps phase 5) ----
        with nc.named_scope("phase4_pc"):
            pc_act = work.tile([64, 9, S], mybir.dt.bfloat16, tag="pcact", bufs=1)
            nc.scalar.activation(pc_act, ppc, RELU, bias=pcbt)
            # H'' bounce: scr_h [e, kk, g, s].  Single batched scatter/gather
            # (was 8+8 DMA instructions at ~0.8us serial issue each).
            dst = bass.AP(scr_h, 0, [[S, 8], [9 * 8 * S, 8], [8 * S, 9], [1, S]])
            nc.scalar.dma_start(out=dst, in_=pc_act[:, :, :])
            H_t = work.tile([72, 8, S], mybir.dt.bfloat16, tag="Ht", bufs=1)
            src = bass.AP(scr_h, 0, [[9 * 8 * S, 8], [8 * S, 9], [S, 8], [1, S]])
            nc.scalar.dma_start(out=H_t[:, :, :], in_=src)

            drip(2)

        # ---- phase 5: drain remaining units (overlaps H bounce) ----
        with nc.named_scope("phase5_fc"):
            drip(max(0, units_len() - 14))

        # ---- phase 6: dynamic filter tail (all f32) ----
        with nc.named_scope("phase6_tail"):
            drip(7)  # keep the PE fed while the H bounce completes
            # stage 1: t'' [(c,d) 2x128, (g,s) 128]
            tps = ps.tile([128, 2, 8, S], F32, tag="fc", bufs=2)
            for ch in range(2):
                nc.tensor.matmul(tps[:, ch, :, :], bprime_t[:, ch * 128:(ch + 1) * 128],
                                 H_t, start=True, stop=True)
            t_sb = work.tile([128, 2, 8, S], mybir.dt.bfloat16, tag="tsb", bufs=1)
            nc.vector.tensor_copy(t_sb, tps)
            for ch in range(2):
                dst = bass.AP(scr_t, ch * 128 * 8 * S, [[8 * S, 128], [S, 8], [1, S]])
                nc.scalar.dma_start(out=dst, in_=t_sb[:, ch, :, :])
            R_t = work.tile([128, 2, 8, S], mybir.dt.bfloat16, tag="Rt", bufs=1)
            for ch in range(2):
                # R_t[gq*32+c, ch, d, s] = scr_t[(c,d), (ch*4+gq, s)]
                src = bass.AP(scr_t, ch * 4 * S,
                              [[S, 4], [8 * 8 * S, 32], [8 * S, 8], [1, S]])
                nc.scalar.dma_start(out=R_t[:, ch, :, :], in_=src)
            drip(units_len())  # L transposes etc. run during the R bounce
            # stage 2: out2 [8(f), 8(d), S]
            o2 = ps.tile([8, 8, S], F32, tag="fc", bufs=2)
            for s in range(S):
                for ch in range(2):
                    nc.tensor.matmul(o2[:, :, s], L_sb[:, ch, :, s], R_t[:, ch, :, s],
                                     start=(ch == 0), stop=(ch == 1))
            feat = work.tile([8, 8, S], mybir.dt.bfloat16, tag="feat", bufs=1)
            nc.vector.tensor_add(feat, o2, bb)
            nc.vector.tensor_relu(feat, feat)
            xps = ps.tile([S, 16], F32, tag="fc", bufs=2)
            for d in range(8):
                nc.tensor.matmul(xps, feat[:, d, :], c2wt_t[:, d, :],
                                 start=(d == 0), stop=(d == 7))
            x_sb = work.tile([S, 16], F32, tag="xsb", bufs=1)
            nc.vector.tensor_add(x_sb, xps, c2bbc)
            dma(out=out_d.ap(), in_=x_sb)

    nc.compile()
    return nc


def _prep_inputs(inputs, cfg):
    """Host-side: shard activations, rearrange weights into tile layouts."""
    i = {k: np.asarray(v, dtype=np.float32) for k, v in inputs.items()}
    cnp = _DT_NP[cfg["conv"]]
    fnp = _DT_NP[cfg["fc"]]
    pnp = _DT_NP[cfg["pc"]]

    obs = i["obs_encoding"].reshape(B, 16384)
    image = i["patch"][:, 0]   # [B,24,24]
    inhand = i["patch"][:, 1]  # [B,24,24]

    # conv1 im2col: [B, 9, 484]
    sw = np.lib.stride_tricks.sliding_window_view(image, (3, 3), axis=(1, 2))
    im1 = sw.transpose(0, 3, 4, 1, 2).reshape(B, 9, 484)
    # ih conv1 im2col (stride 2, pad 1): [B, 9, 144]
    ip = np.pad(inhand, ((0, 0), (1, 1), (1, 1)))
    swi = np.lib.stride_tricks.sliding_window_view(ip, (3, 3), axis=(1, 2))[:, ::2, ::2]
    ihim = swi.transpose(0, 3, 4, 1, 2).reshape(B, 9, 144)

    def conv_w_t(w, nchunk):  # [O, I, 3, 3] -> [128, nchunk, 9, O]
        O, I = w.shape[0], w.shape[1]
        return np.ascontiguousarray(
            w.reshape(O, nchunk, 128, 9).transpose(2, 1, 3, 0))

    # conv2 weights -> [ci_i(128), oc_chunk(4), ci_chunk(2), 9, oc_i(128)]
    w2 = i["enc_w2"].reshape(4, 128, 2, 128, 9)
    w2t = np.ascontiguousarray(w2.transpose(3, 0, 2, 4, 1))

    obsT_full = np.ascontiguousarray(obs.T)  # [16384, 128]

    shared = {
        "w1r": np.ascontiguousarray(i["enc_w1"].reshape(256, 9).T).astype(cnp),
        "w2t": w2t.astype(cnp),
        "w3t": np.ascontiguousarray(
            i["enc_w3"].reshape(8, 128, 4, 128, 9).transpose(0, 3, 2, 4, 1)).astype(cnp),
        "pcwt": conv_w_t(i["pc_w"], 8).astype(pnp),
        "ihw1r": np.ascontiguousarray(i["ih_w1"].reshape(64, 9).T).astype(cnp),
        "ihw2t": np.ascontiguousarray(
            i["ih_w2"].reshape(128, 64, 9).transpose(1, 2, 0)).astype(cnp),
        # feature order f' = sp*128 + ci  (ih flatten is ci*36 + sp)
        "ihfcwt": np.ascontiguousarray(
            i["ih_fc_w"].reshape(512, 128, 36).transpose(2, 1, 0).reshape(4608, 512)
        ).astype(fnp),
        "esw2t": np.ascontiguousarray(i["es_w2"].T).astype(fnp),
        "dfwt": np.ascontiguousarray(i["df_w"].T).astype(fnp),
        # basis [c,d,e,kh,kw] -> [(e,kk), (c,d)]
        "bprime": np.ascontiguousarray(
            i["basis"].reshape(32, 8, 8, 9).transpose(2, 3, 0, 1).reshape(72, 256)
        ).astype(ml_dtypes.bfloat16),
        # c2_w [o, f*8+d] -> [f, d, o]
        "c2wt": np.ascontiguousarray(
            i["c2_w"].reshape(16, 8, 8).transpose(1, 2, 0)).astype(ml_dtypes.bfloat16),
    }
    # conv biases packed per-partition: [b1(2) b2(4) b3(8) ihb2(1) pcb(1) ihb1(1)]
    blobA = np.zeros((128, 17), np.float32)
    blobA[:, 0:2] = i["enc_b1"].reshape(2, 128).T
    blobA[:, 2:6] = i["enc_b2"].reshape(4, 128).T
    blobA[:, 6:14] = i["enc_b3"].reshape(8, 128).T
    blobA[:, 14] = i["ih_b2"]
    blobA[0:64, 15] = i["pc_b"]
    blobA[0:64, 16] = i["ih_b1"]
    shared["bblobA"] = blobA
    # S-broadcast FC biases: [esb1(1024) esb2(512) ihfcb(512) c2b(16) dfb(2056)]
    blobB = np.concatenate([i["es_b1"], i["es_b2"], i["ih_fc_b"], i["c2_b"],
                            i["df_b"]]).astype(np.float32)
    shared["bblobB"] = np.ascontiguousarray(np.broadcast_to(blobB, (S, 4120)))
    esw1_full = np.ascontiguousarray(i["es_w1"].T)  # [16384, 1024]
    in_maps = []
    for c in range(NCORES):
        sl = slice(c * S, (c + 1) * S)
        ksl = slice(c * KSH, (c + 1) * KSH)
        m = dict(shared)
        # es1 K-shard: obs rows for ALL samples, [p, kc, s] contiguous layout
        m["obsT"] = np.ascontiguousarray(
            obsT_full[ksl].reshape(KSH // 128, 128, B).transpose(1, 0, 2)).astype(fnp)
        m["esw1t"] = np.ascontiguousarray(esw1_full[ksl]).astype(fnp)
        m["im1"] = np.ascontiguousarray(im1[sl].transpose(1, 0, 2)).astype(cnp)
        m["ihim"] = np.ascontiguousarray(ihim[sl].transpose(1, 0, 2)).astype(cnp)
        in_maps.append(m)
    return in_maps


_CACHE = {}


def _get_nc(cfg):
    key = tuple(sorted(cfg.items()))
    if key not in _CACHE:
        _CACHE[key] = build(cfg)
    return _CACHE[key]


def run(inputs, cfg=None, trace=False):
    cfg = cfg or DEFAULT_CFG
    nc = _get_nc(cfg)
    in_maps = _prep_inputs(inputs, cfg)
    res = run_bass_kernel_spmd(nc, in_maps, list(range(NCORES)), trace=trace)
    out = np.concatenate([res.results[c]["out"] for c in range(NCORES)], axis=0)
    return out.astype(np.float32), res


def kernel(**inputs) -> np.ndarray:
    out, _ = run(inputs)
    return out

